# revision 23
# baseline (speedup 1.0000x reference)
"""Trainium2 Bass kernel for nn_PoissonMatchLoss (retrieval_knn).

Computes, for inputs x[N,D] (hypotheses) and t[M,D] (measurements):
    L_ij = sum_d x[i,d] - sum_d t[j,d]*log(x[i,d]+eps) + sum_d stirling(t[j,d])
    loss = mean_i min_j L_ij ;  match_i = argmin_j L_ij

Sharding: hypothesis rows across 8 cores (N/8 rows each), target replicated.

Per-core device algorithm:
  P_ij = -cross_ij + s_st[j] is produced by one PE matmul with an augmented
  contraction: lhsT = [-log(x+eps)^T ; ones-row], rhs = [t^T ; s_st-row],
  K = 181 (two K-chunks 128+53), fp32r (full-rate fp32).
  s_st (Stirling sums) is computed in natural layout: ACT Ln/Relu/Sign +
  two GPSIMD scalar_tensor_tensor passes with sum accumulators, using
    stirling(t) = (t+0.5)*l - (t - 0.5*ln(2pi))*[t>1],  l = relu(ln t).
  Per 1536-col block (outer loop) and 128-row tile (inner loop): fused DVE
  tensor_tensor_reduce does PSUM->SBUF copy plus min-reduce (v_b per row);
  a 2x-rate tensor_scalar(is_le) emits a u8 equality mask, DMA'd out per
  block. s_in comes from a row-sum of x.
Host: picks the first block achieving the row min, locates the first set
mask bit inside it (== first-occurrence argmin of the device's own values),
adds s_in and means the loss.
"""

import sys

for _p in ("/root/.axon_site", "/root/.axon_site/_ro/trn_rl_repo",
           "/root/.axon_site/_ro/pypackages", "/opt/trn_rl_repo", "/opt/pypackages"):
    if _p not in sys.path:
        sys.path.append(_p)

from contextlib import ExitStack

import numpy as np

import concourse.bacc as bacc
import concourse.mybir as mybir
from concourse.masks import make_identity
from concourse.tile import TileContext

F32 = mybir.dt.float32
F32R = mybir.dt.float32r
U8 = mybir.dt.uint8
ALU = mybir.AluOpType
ACTF = mybir.ActivationFunctionType
AX_X = mybir.AxisListType.X

N_CORES = 8
N_FULL, M_FULL, D_FULL = 10000, 10000, 180
EPS = 1e-8
C_ST = 0.9189385332046727  # 0.5*ln(2*pi)
# fp32r (11-bit mantissa) worst-case |L_device - L_exact| is ~5.6 for this
# data; candidates within MARGIN of the device row-min are re-scored exactly
# on the host, so argmin survives fp32r rounding.
MARGIN = 12.0


def _csubs(width):
    """512-aligned free-dim chunks (each matmul dest inside one PSUM bank)."""
    return [(o, min(512, width - o)) for o in range(0, width, 512)]


def _quads(nrows):
    """Split row range into DMA batches: full 512-row quads, then a tail of
    full 128-tiles, then a final partial tile. Returns (row0, ntiles, rows)."""
    out = []
    off = 0
    while nrows - off >= 512:
        out.append((off, 4, 512))
        off += 512
    if nrows - off >= 128:
        nt = (nrows - off) // 128
        out.append((off, nt, nt * 128))
        off += nt * 128
    if nrows - off > 0:
        out.append((off, 1, nrows - off))
    return out


def build_nc(R=N_FULL // N_CORES, M=M_FULL, D=D_FULL, BLK=1024, stir_pool=False,
             mask_dve_min_r=9, loop_iters=0):
    """Emit the per-core Bass program. Same program runs SPMD on all cores."""
    assert D == 180
    K2 = D - 128 + 1  # second K-chunk partitions: dims 128..179 plus ones row
    nrt = (R + 127) // 128          # row tiles
    ntt = (M + 127) // 128          # target tiles
    tpg = BLK // 128                # target tiles per block
    nblk = (M + BLK - 1) // BLK     # column blocks

    nc = bacc.Bacc("TRN2", target_bir_lowering=False, debug=False)
    x_d = nc.dram_tensor("x", [R, D], F32, kind="ExternalInput").ap()
    t_d = nc.dram_tensor("t", [M, D], F32, kind="ExternalInput").ap()
    v_d = nc.dram_tensor("v_out", [R, nblk], F32, kind="ExternalOutput").ap()
    s_d = nc.dram_tensor("s_out", [128, nrt], F32, kind="ExternalOutput").ap()
    m_d = nc.dram_tensor("m_out", [R, nblk * BLK], U8, kind="ExternalOutput").ap()

    stir_eng = nc.gpsimd if stir_pool else nc.vector

    with ExitStack() as ctx:
        tc = ctx.enter_context(TileContext(nc))
        consts = ctx.enter_context(tc.tile_pool(name="consts", bufs=1))
        lh_pool = ctx.enter_context(tc.tile_pool(name="lh", bufs=1))
        tgt_pool = ctx.enter_context(tc.tile_pool(name="tgt", bufs=1))
        ld_pool = ctx.enter_context(tc.tile_pool(name="ld", bufs=3))
        st_pool = ctx.enter_context(tc.tile_pool(name="st", bufs=2))
        col_pool = ctx.enter_context(tc.tile_pool(name="col", bufs=1))
        mn_pool = ctx.enter_context(tc.tile_pool(name="mn", bufs=3))
        mk_pool = ctx.enter_context(tc.tile_pool(name="mk", bufs=2))
        res_pool = ctx.enter_context(tc.tile_pool(name="res", bufs=1))
        ps_main = ctx.enter_context(tc.tile_pool(name="psmain", bufs=3, space="PSUM"))
        ps_tp = ctx.enter_context(tc.tile_pool(name="pstp", bufs=1, space="PSUM"))

        ident = consts.tile([128, 128], F32)
        make_identity(nc, ident)
        zcol = consts.tile([128, 1], F32)
        nc.gpsimd.memset(zcol, 0.0)
        eps_c = consts.tile([128, 1], F32)
        nc.gpsimd.memset(eps_c, EPS)
        tiny_c = consts.tile([128, 1], F32)
        nc.gpsimd.memset(tiny_c, 1e-30)
        ones_row = consts.tile([1, 128], F32)
        nc.gpsimd.memset(ones_row, 1.0)

        # Persistent transposed operands.
        tgtT1 = [tgt_pool.tile([128, BLK], F32, tag=f"t1_{b}", name=f"t1_{b}")
                 for b in range(nblk)]
        tgtT2 = [tgt_pool.tile([K2, BLK], F32, tag=f"t2_{b}", name=f"t2_{b}")
                 for b in range(nblk)]
        lh1 = [lh_pool.tile([128, 128], F32, tag=f"lh1_{r}", name=f"lh1_{r}")
               for r in range(nrt)]
        lh2 = [lh_pool.tile([K2, 128], F32, tag=f"lh2_{r}", name=f"lh2_{r}")
               for r in range(nrt)]
        resv = [res_pool.tile([128, nblk], F32, tag=f"resv_{r}", name=f"resv_{r}")
                for r in range(nrt)]

        loop_cm = tc.For_i(0, loop_iters, 1) if loop_iters else None
        if loop_cm is not None:
            ctx.enter_context(loop_cm)

        # ---- hypothesis side: load, row sums, -log^T ----
        for row0, nt, rows in _quads(R):
            q0 = row0 // 128
            xa = ld_pool.tile([128, 4, D], F32, tag="xa")
            nc.sync.dma_start(
                out=xa[:128, :nt, :] if rows >= 128 else xa[:rows, :1, :],
                in_=x_d[row0:row0 + rows, :].rearrange("(q p) d -> p q d", q=nt)
                if rows >= 128 else x_d[row0:row0 + rows, :])
            prq = min(128, rows)
            sin = ld_pool.tile([128, 4], F32, tag="sin")
            nc.vector.tensor_reduce(out=sin[:prq, :nt], in_=xa[:prq, :nt, :],
                                    axis=AX_X, op=ALU.add)
            nc.sync.dma_start(out=s_d[:prq, q0:q0 + nt], in_=sin[:prq, :nt])
            lx = ld_pool.tile([128, 4, D], F32, tag="lx")
            nc.scalar.activation(out=lx[:prq, :nt, :], in_=xa[:prq, :nt, :],
                                 func=ACTF.Ln, bias=eps_c[:prq])
            for q in range(nt):
                r = q0 + q
                pr = min(128, R - r * 128)
                tp1 = ps_tp.tile([128, 512], F32, tag="tp1")
                nc.tensor.transpose(out=tp1[:, :pr], in_=lx[:pr, q, 0:128],
                                    identity=ident[:pr, :pr])
                nc.scalar.activation(out=lh1[r][:, :pr].bitcast(F32R),
                                     in_=tp1[:, :pr], func=ACTF.Copy, scale=-1.0)
                tp2 = ps_tp.tile([52, 512], F32, tag="tp2")
                nc.tensor.transpose(out=tp2[:D - 128, :pr], in_=lx[:pr, q, 128:D],
                                    identity=ident[:pr, :pr])
                nc.scalar.activation(out=lh2[r][0:D - 128, :pr].bitcast(F32R),
                                     in_=tp2[:D - 128, :pr],
                                     func=ACTF.Copy, scale=-1.0)
                nc.sync.dma_start(out=lh2[r][D - 128:K2, :], in_=ones_row[:, :])

        # ---- target side: t^T plus Stirling column sums ----
        scolA = [col_pool.tile([128, tpg], F32, tag=f"scol_{g}", name=f"scol_{g}")
                 for g in range(nblk)]
        for g in range(nblk):
            nc.gpsimd.memset(scolA[g], 0.0)

        def prep_block(bb):
            base = bb * BLK
            width = min(BLK, M - base)
            for off0, nt, rows in _quads(width):
                _prep_quad(base + off0, nt, rows)
            # s_st columns -> row 52 of tgtT2[bb]
            ng = min(tpg, ntt - bb * tpg)
            tps = ps_tp.tile([128, 512], F32, tag="tp1", name=f"tps_{bb}")
            nc.tensor.transpose(out=tps[:ng, :128], in_=scolA[bb][:, :ng],
                                identity=ident)
            sT = col_pool.tile([128, 128], F32, tag="sT", name=f"sT_{bb}")
            nc.scalar.copy(out=sT[:ng, :].bitcast(F32R), in_=tps[:ng, :128])
            nc.sync.dma_start(out=tgtT2[bb][K2 - 1:K2, 0:ng * 128], in_=sT[:ng, :])

        def _prep_quad(row0, nt, rows):
            q0 = row0 // 128
            prq = min(128, rows)
            ta = ld_pool.tile([128, 4, D], F32, tag="ta")
            nc.sync.dma_start(
                out=ta[:128, :nt, :] if rows >= 128 else ta[:rows, :1, :],
                in_=t_d[row0:row0 + rows, :].rearrange("(q p) d -> p q d", q=nt)
                if rows >= 128 else t_d[row0:row0 + rows, :])
            # stirling(t) = (t+0.5)*l - (t-C_ST)*sign(l), l = relu(ln t)
            lnt = st_pool.tile([128, 4, D], F32, tag="lnt")
            nc.scalar.activation(out=lnt[:prq, :nt, :], in_=ta[:prq, :nt, :],
                                 func=ACTF.Ln, bias=tiny_c[:prq])
            lrl = st_pool.tile([128, 4, D], F32, tag="lrl")
            nc.scalar.activation(out=lrl[:prq, :nt, :], in_=lnt[:prq, :nt, :],
                                 func=ACTF.Relu)
            msk = st_pool.tile([128, 4, D], F32, tag="msk")
            nc.scalar.activation(out=msk[:prq, :nt, :], in_=lrl[:prq, :nt, :],
                                 func=ACTF.Sign)
            jb = st_pool.tile([128, D], F32, tag="jb")
            jc = st_pool.tile([128, D], F32, tag="jc")
            sB = st_pool.tile([128, 4], F32, tag="sB")
            sC = st_pool.tile([128, 4], F32, tag="sC")
            # transposes: batch 4 subtile transposes into one psum tile
            tpq1 = ps_tp.tile([128, 512], F32, tag="tp1")
            tpq2 = ps_tp.tile([52, 512], F32, tag="tp2")
            for q in range(nt):
                tt = q0 + q
                pt = min(128, M - tt * 128)
                nc.tensor.transpose(out=tpq1[:, q * 128:q * 128 + pt],
                                    in_=ta[:pt, q, 0:128], identity=ident[:pt, :pt])
                nc.tensor.transpose(out=tpq2[:D - 128, q * 128:q * 128 + pt],
                                    in_=ta[:pt, q, 128:D], identity=ident[:pt, :pt])
                stir_eng.scalar_tensor_tensor(
                    out=jb[:pt], in0=ta[:pt, q, :], scalar=0.5, in1=lrl[:pt, q, :],
                    op0=ALU.add, op1=ALU.mult, accum_out=sB[:pt, q:q + 1])
                stir_eng.scalar_tensor_tensor(
                    out=jc[:pt], in0=ta[:pt, q, :], scalar=C_ST, in1=msk[:pt, q, :],
                    op0=ALU.subtract, op1=ALU.mult, accum_out=sC[:pt, q:q + 1])
                g, ig = tt // tpg, tt % tpg
                nc.gpsimd.tensor_sub(out=scolA[g][:pt, ig:ig + 1],
                                     in0=sB[:pt, q:q + 1], in1=sC[:pt, q:q + 1])
            # copy batched transposes into block tiles
            b = row0 // BLK
            c0 = row0 - b * BLK
            nc.scalar.copy(out=tgtT1[b][:, c0:c0 + rows].bitcast(F32R),
                           in_=tpq1[:, :rows])
            nc.scalar.copy(out=tgtT2[b][0:D - 128, c0:c0 + rows].bitcast(F32R),
                           in_=tpq2[:D - 128, :rows])

        # ---- main: per block, emit target-prep then matmul+min+mask ----
        for b in range(nblk):
            prep_block(b)
            bw = min(BLK, M - b * BLK)
            mkb = mk_pool.tile([128, nrt, BLK], U8, tag="mkb")
            for r in range(nrt):
                pr = min(128, R - r * 128)
                ps = ps_main.tile([128, BLK], F32, tag="ps")
                for off, w in _csubs(bw):
                    nc.tensor.matmul(out=ps[:pr, off:off + w],
                                     lhsT=lh1[r][:, :pr].bitcast(F32R),
                                     rhs=tgtT1[b][:, off:off + w].bitcast(F32R),
                                     start=True, stop=False)
                    nc.tensor.matmul(out=ps[:pr, off:off + w],
                                     lhsT=lh2[r][:, :pr].bitcast(F32R),
                                     rhs=tgtT2[b][:, off:off + w].bitcast(F32R),
                                     start=False, stop=True)
                nc.vector.tensor_reduce(out=resv[r][:pr, b:b + 1],
                                        in_=ps[:pr, :bw], axis=AX_X, op=ALU.min)
                if r < mask_dve_min_r:
                    nbias = mn_pool.tile([128, 1], F32, tag="nbias")
                    nc.vector.tensor_scalar(out=nbias[:pr],
                                            in0=resv[r][:pr, b:b + 1],
                                            scalar1=-1.0, scalar2=-MARGIN,
                                            op0=ALU.mult, op1=ALU.add)
                    nc.scalar.activation(out=mkb[:pr, r, :bw], in_=ps[:pr, :bw],
                                         func=ACTF.Sign, bias=nbias[:pr])
                else:
                    nc.vector.tensor_scalar(out=mkb[:pr, r, :bw],
                                            in0=ps[:pr, :bw],
                                            scalar1=resv[r][:pr, b:b + 1],
                                            scalar2=MARGIN, op0=ALU.subtract,
                                            op1=ALU.is_gt)
            # store masks: full row-tiles in one DMA, partial tail separately
            nfull = R // 128
            if nfull:
                nc.sync.dma_start(
                    out=m_d[0:nfull * 128, b * BLK:b * BLK + bw]
                    .rearrange("(r p) c -> p r c", p=128),
                    in_=mkb[:, :nfull, :bw])
            if R % 128:
                pr = R % 128
                nc.sync.dma_start(
                    out=m_d[nfull * 128:R, b * BLK:b * BLK + bw],
                    in_=mkb[:pr, nfull, :bw])
        for r in range(nrt):
            pr = min(128, R - r * 128)
            nc.sync.dma_start(out=v_d[r * 128:r * 128 + pr, :], in_=resv[r][:pr, :])

    nc.compile()
    return nc


_CACHE = {}


def _decode_core(v, s, mk, blk, lx64, t64, s_st64):
    """Host decode for one core: candidate columns (device values within
    MARGIN of the row min, fp32r noise bound) are re-scored exactly; the
    refined argmin/min replace the fp32r-noised device values.
    s is [128, nrt] column-major (s[p, r] = row r*128+p)."""
    R, nblk = v.shape
    M = t64.shape[0]
    vmin = v.min(axis=1)
    blk_ok = v <= vmin[:, None] + MARGIN          # [R, nblk]
    mkr = mk.reshape(R, nblk, blk) != 1
    cand = mkr & blk_ok[:, :, None]
    ii, bb, jj = np.nonzero(cand)
    j = bb * blk + jj
    keep = j < M
    ii, j = ii[keep], j[keep]
    # exact scores for candidate pairs
    sc = s_st64[j] - np.einsum("kd,kd->k", lx64[ii], t64[j])
    # per-row min with first-occurrence tie rule: (ii, j) is sorted by (ii, j)
    # already (np.nonzero order), so a stable lexsort on (score) within rows
    # keeps the smallest j among equal scores first.
    order = np.lexsort((j, sc, ii))
    ii_o, j_o, sc_o = ii[order], j[order], sc[order]
    first = np.unique(ii_o, return_index=True)[1]
    assert first.shape[0] == R, "every row must have at least one candidate"
    match = j_o[first].astype(np.int64)
    best = sc_o[first]
    s_in = s.T.reshape(-1)[:R]
    lossv = best + s_in.astype(np.float64)
    return match, lossv


def kernel(input, target):
    from concourse.bass_utils import run_bass_kernel_spmd

    input = np.ascontiguousarray(input, dtype=np.float32)
    target = np.ascontiguousarray(target, dtype=np.float32)
    N = input.shape[0]
    R = N // N_CORES
    BLK = 1024

    if "nc" not in _CACHE:
        _CACHE["nc"] = build_nc(R=R, M=target.shape[0], D=target.shape[1], BLK=BLK)
    nc = _CACHE["nc"]

    in_maps = [{"x": input[c * R:(c + 1) * R], "t": target} for c in range(N_CORES)]
    res = run_bass_kernel_spmd(nc, in_maps, list(range(N_CORES)))
    outs = res.results

    t64 = target.astype(np.float64)
    lx64 = np.log(input.astype(np.float64) + EPS)
    st = np.where(t64 > 1,
                  t64 * np.log(np.maximum(t64, 1.0)) - t64
                  + 0.5 * np.log(2 * np.pi * np.maximum(t64, 1.0)), 0.0)
    s_st64 = st.sum(1)

    match = np.empty(N, dtype=np.int32)
    lossv = np.empty(N, dtype=np.float64)
    for c in range(N_CORES):
        m_c, l_c = _decode_core(outs[c]["v_out"], outs[c]["s_out"],
                                outs[c]["m_out"], BLK,
                                lx64[c * R:(c + 1) * R], t64, s_st64)
        match[c * R:(c + 1) * R] = m_c
        lossv[c * R:(c + 1) * R] = l_c
    loss = np.float32(lossv.mean())
    return (loss, match)
